# revision 8
# baseline (speedup 1.0000x reference)
"""Trainium2 Bass kernel for nn_ODEModel (single 3/8-rule RK4 step of a
2-layer MLP ODE function), data-parallel across 8 NeuronCores.

Math (per row of x, dt=1):
    f(y) = W2 @ relu(W1 @ y + b1) + b2
    k1 = f(x); k2 = f(x + k1/3); k3 = f(x + k2 - k1/3); k4 = f(x + k1 - k2 + k3)
    out = x + (k1 + 3*(k2 + k3) + k4) / 8

Device strategy (per core, shard of B rows):
  - Transposed activation layout: tiles are [feat/hid partitions, rows free].
    Host pre-transposes x into xT [128, B] so every DMA is contiguous.
  - Per block of R=512 rows: z = W1@y via 2 bf16 matmuls (hid chunks) into a
    fused [128, 1024] PSUM tile; h = relu(z) is ONE activation op on ScalarE;
    k_s = W2@h_s via 2 accumulating matmuls; the final RK4 combination
    sum((c_s*W2) @ h_s) is accumulated in PSUM across all 4 stages with
    host-pre-scaled bf16 weight copies (c = 1/8, 3/8, 3/8, 1/8).
  - y combinations are fused scalar_tensor_tensor ops: y = (k * c) + prev.
    k1-derived ones (y2, xm, v) run on GpSimd from an SBUF copy of k1;
    the PSUM-reading ones (y3, w, y4, out) run on VectorE.
  - The emission order is software-pipelined with a 2-stage skew between
    consecutive row blocks so the RK4 serial dependency chain of one block
    hides under the engine work of its neighbors.

b1/b2 are zero in the reference's setup_inputs; a bias-correct variant
(per-chunk relus with per-partition bias, bias terms folded host-side)
is built only when a nonzero bias is actually passed.
"""

import ml_dtypes
import numpy as np

import concourse.bass as bass
import concourse.bacc as bacc_mod
import concourse.mybir as mybir
from concourse.bass_utils import run_bass_kernel_spmd
from concourse.tile import TileContext

F32 = mybir.dt.float32
BF16 = mybir.dt.bfloat16
AF = mybir.ActivationFunctionType
ALU = mybir.AluOpType

N_CORES = 8
D = 128          # IN_DIM
H = 256          # HID
R = 512          # rows per block
BATCH = 262144
B_LOCAL = BATCH // N_CORES          # 32768 rows per core
NBLK = B_LOCAL // R                 # 64 blocks per core

# y2/xm/v on GpSimd (from an SBUF k1 copy) to unload VectorE
GPSIMD_STT = True


def build(nblk: int, with_bias: bool) -> bass.Bass:
    nc = bacc_mod.Bacc(None, target_bir_lowering=False, debug=False)
    B = nblk * R

    xT = nc.declare_dram_parameter("xT", [D, B], F32, isOutput=False)
    w1t = nc.declare_dram_parameter("w1t", [D, H], BF16, isOutput=False)
    # w2ts: [0] = W2.T, [1] = W2.T/8, [2] = 3*W2.T/8   (each [H, D], bf16)
    w2ts = nc.declare_dram_parameter("w2ts", [3, H, D], BF16, isOutput=False)
    if with_bias:
        # biasesT[p, s*2+c] = (b1 + cfold[s]*(W1@b2))[c*128+p], cfold=(0,1/3,2/3,1)
        biases = nc.declare_dram_parameter("biases", [D, 8], F32, isOutput=False)
        b2col = nc.declare_dram_parameter("b2col", [D, 1], F32, isOutput=False)
    outT = nc.declare_dram_parameter("outT", [D, B], F32, isOutput=True)

    with TileContext(nc) as tc:
        with (
            tc.tile_pool(name="wpool", bufs=1) as wpool,
            tc.tile_pool(name="xpool", bufs=5) as xpool,
            tc.tile_pool(name="xbpool", bufs=4) as xbpool,
            tc.tile_pool(name="ypool", bufs=4) as ypool,
            tc.tile_pool(name="hpool", bufs=4) as hpool,
            tc.tile_pool(name="apool", bufs=3) as apool,
            tc.tile_pool(name="opool", bufs=4) as opool,
            tc.tile_pool(name="zpool", bufs=2, space="PSUM") as zpool,
            tc.tile_pool(name="kpool", bufs=2, space="PSUM") as kpool,
            tc.tile_pool(name="gpool", bufs=2, space="PSUM") as gpool,
        ):
            # ---- one-time weight/bias loads ----
            w1t_sb = wpool.tile([D, H], BF16, tag="w1t")
            nc.sync.dma_start(out=w1t_sb, in_=w1t[:, :])
            w2_sb = []  # [variant][chunk] -> [128, 128] bf16
            for v in range(3):
                per_chunk = []
                for c in range(2):
                    t = wpool.tile([D, D], BF16, tag=f"w2v{v}c{c}")
                    nc.sync.dma_start(out=t, in_=w2ts[v, c * D : (c + 1) * D, :])
                    per_chunk.append(t)
                w2_sb.append(per_chunk)
            if with_bias:
                bias_sb = wpool.tile([D, 8], F32, tag="biases")
                nc.sync.dma_start(out=bias_sb, in_=biases[:, :])
                b2_sb = wpool.tile([D, 1], F32, tag="b2col")
                nc.sync.dma_start(out=b2_sb, in_=b2col[:, :])

            # g-accumulation weight variant per stage: 1/8, 3/8, 3/8, 1/8
            gvar = (1, 2, 2, 1)

            def mlp_z(y_sb, stage):
                """z = W1 @ y; h = relu(z + bias_s). Returns h tile."""
                z = zpool.tile([D, 2 * R], F32, tag="z")
                for c in range(2):
                    nc.tensor.matmul(
                        out=z[:, c * R : (c + 1) * R],
                        lhsT=w1t_sb[:, c * D : (c + 1) * D],
                        rhs=y_sb[:, :],
                        start=True,
                        stop=True,
                    )
                h = hpool.tile([D, 2 * R], BF16, tag="h")
                if with_bias:
                    for c in range(2):
                        nc.scalar.activation(
                            out=h[:, c * R : (c + 1) * R],
                            in_=z[:, c * R : (c + 1) * R],
                            func=AF.Relu,
                            bias=bias_sb[:, stage * 2 + c : stage * 2 + c + 1],
                        )
                else:
                    nc.scalar.activation(out=h[:, :], in_=z[:, :], func=AF.Relu)
                return h

            def mm_k(h, stage, g, want_k):
                """k_s = W2 @ h_s (PSUM accum over hid chunks) if want_k,
                plus g += (c_s * W2) @ h_s into the block's g accumulator."""
                k = None
                if want_k:
                    k = kpool.tile([D, R], F32, tag="k")
                    for c in range(2):
                        nc.tensor.matmul(
                            out=k[:, :],
                            lhsT=w2_sb[0][c][:, :],
                            rhs=h[:, c * R : (c + 1) * R],
                            start=(c == 0),
                            stop=(c == 1),
                        )
                for c in range(2):
                    nc.tensor.matmul(
                        out=g[:, :],
                        lhsT=w2_sb[gvar[stage]][c][:, :],
                        rhs=h[:, c * R : (c + 1) * R],
                        start=(stage == 0 and c == 0),
                        stop=(stage == 3 and c == 1),
                        skip_group_check=True,
                    )
                return k

            def stt(eng, out_ap, in0, scalar, in1):
                eng.scalar_tensor_tensor(
                    out=out_ap,
                    in0=in0,
                    scalar=scalar,
                    in1=in1,
                    op0=ALU.mult,
                    op1=ALU.add,
                )

            # ---- per-block state and stage closures (for the skew) ----
            st: list[dict] = [dict() for _ in range(nblk)]

            def pre(i):
                s = st[i]
                cols = slice(i * R, (i + 1) * R)
                s["cols"] = cols
                x_t = xpool.tile([D, R], F32, tag="x")
                nc.sync.dma_start(out=x_t, in_=xT[:, cols])
                x_bf = xbpool.tile([D, R], BF16, tag="xbf")
                nc.vector.tensor_copy(out=x_bf[:, :], in_=x_t[:, :])
                s["x"] = x_t
                s["xbf"] = x_bf

            def s1(i):
                s = st[i]
                g = gpool.tile([D, R], F32, tag="g")
                s["g"] = g
                h1 = mlp_z(s["xbf"], 0)
                k1 = mm_k(h1, 0, g, True)
                # k1 combos
                y2 = ypool.tile([D, R], BF16, tag="y")
                xm = apool.tile([D, R], BF16, tag="xm")
                v = apool.tile([D, R], BF16, tag="v")
                if GPSIMD_STT:
                    # k1 scaled copies on DVE, plain adds on GpSimd
                    k1e3 = apool.tile([D, R], BF16, tag="k1e3")
                    nc.vector.tensor_scalar_mul(k1e3[:, :], k1[:, :], 1.0 / 3.0)
                    k1e = apool.tile([D, R], BF16, tag="k1e")
                    nc.vector.tensor_copy(out=k1e[:, :], in_=k1[:, :])
                    nc.gpsimd.tensor_tensor(
                        y2[:, :], s["xbf"][:, :], k1e3[:, :], ALU.add
                    )
                    nc.gpsimd.tensor_tensor(
                        xm[:, :], s["xbf"][:, :], k1e3[:, :], ALU.subtract
                    )
                    nc.gpsimd.tensor_tensor(
                        v[:, :], s["xbf"][:, :], k1e[:, :], ALU.add
                    )
                else:
                    stt(nc.vector, y2[:, :], k1[:, :], 1.0 / 3.0, s["x"][:, :])
                    stt(nc.vector, xm[:, :], k1[:, :], -1.0 / 3.0, s["x"][:, :])
                    stt(nc.vector, v[:, :], k1[:, :], 1.0, s["x"][:, :])
                s["y2"], s["xm"], s["v"] = y2, xm, v

            def s2(i):
                s = st[i]
                h2 = mlp_z(s["y2"], 1)
                k2 = mm_k(h2, 1, s["g"], True)
                y3 = ypool.tile([D, R], BF16, tag="y")
                stt(nc.vector, y3[:, :], k2[:, :], 1.0, s["xm"][:, :])
                w = apool.tile([D, R], BF16, tag="w")
                stt(nc.vector, w[:, :], k2[:, :], -1.0, s["v"][:, :])
                s["y3"], s["w"] = y3, w

            def s3(i):
                s = st[i]
                h3 = mlp_z(s["y3"], 2)
                k3 = mm_k(h3, 2, s["g"], True)
                y4 = ypool.tile([D, R], BF16, tag="y")
                stt(nc.vector, y4[:, :], k3[:, :], 1.0, s["w"][:, :])
                s["y4"] = y4

            def s4(i):
                s = st[i]
                h4 = mlp_z(s["y4"], 3)
                mm_k(h4, 3, s["g"], False)
                o = opool.tile([D, R], F32, tag="o")
                if with_bias:
                    nc.vector.scalar_tensor_tensor(
                        out=o[:, :],
                        in0=s["g"][:, :],
                        scalar=b2_sb[:, :],
                        in1=s["x"][:, :],
                        op0=ALU.add,
                        op1=ALU.add,
                    )
                else:
                    stt(nc.vector, o[:, :], s["g"][:, :], 1.0, s["x"][:, :])
                nc.sync.dma_start(out=outT[:, s["cols"]], in_=o[:, :])
                st[i] = {}  # release references

            # ---- skewed emission: 2-stage offset between adjacent blocks ----
            # order: ..., S3(i), S1(i+1), S4(i), S2(i+1), S3(i+1), S1(i+2), ...
            pre(0)
            s1(0)
            if nblk > 1:
                pre(1)
            s2(0)
            if nblk == 1:
                s3(0)
                s4(0)
            else:
                for i in range(nblk - 1):
                    s3(i)
                    s1(i + 1)
                    if i + 2 < nblk:
                        pre(i + 2)
                    s4(i)
                    s2(i + 1)
                s3(nblk - 1)
                s4(nblk - 1)

    nc.finalize()
    return nc


_cache: dict = {}


def _get_nc(nblk: int, with_bias: bool) -> bass.Bass:
    key = (nblk, with_bias)
    if key not in _cache:
        _cache[key] = build(nblk, with_bias)
    return _cache[key]


def _prep_host(x, W1, b1, W2, b2):
    x = np.asarray(x, dtype=np.float32)
    W1 = np.asarray(W1, dtype=np.float32)
    b1 = np.asarray(b1, dtype=np.float32)
    W2 = np.asarray(W2, dtype=np.float32)
    b2 = np.asarray(b2, dtype=np.float32)

    with_bias = bool(np.any(b1) or np.any(b2))

    xT = np.ascontiguousarray(x.T)                                 # [128, BATCH]
    w1t = np.ascontiguousarray(W1.T).astype(ml_dtypes.bfloat16)    # [128, 256]
    w2t = np.ascontiguousarray(W2.T)                               # [256, 128]
    w2ts = np.stack([w2t, w2t / 8.0, 3.0 * w2t / 8.0]).astype(
        ml_dtypes.bfloat16
    )  # [3, 256, 128]

    shard_maps = []
    for c in range(N_CORES):
        m = {
            "xT": np.ascontiguousarray(xT[:, c * B_LOCAL : (c + 1) * B_LOCAL]),
            "w1t": w1t,
            "w2ts": w2ts,
        }
        if with_bias:
            c1 = W1 @ b2
            bl = np.stack(
                [b1, b1 + c1 / 3.0, b1 + 2.0 * c1 / 3.0, b1 + c1]
            ).astype(np.float32)  # [4, 256]
            # -> [128, 8]: col s*2+c holds bias_s[c*128:(c+1)*128]
            m["biases"] = np.ascontiguousarray(
                bl.reshape(4, 2, D).transpose(2, 0, 1).reshape(D, 8)
            )
            m["b2col"] = b2.reshape(D, 1)
        shard_maps.append(m)
    return shard_maps, with_bias


def run(x, W1, b1, W2, b2, trace: bool = False):
    """Run on the 8 cores; returns (out [BATCH, 128] fp32, BassKernelResults)."""
    shard_maps, with_bias = _prep_host(x, W1, b1, W2, b2)
    nc = _get_nc(NBLK, with_bias)
    res = run_bass_kernel_spmd(
        nc, shard_maps, core_ids=list(range(N_CORES)), trace=trace
    )
    outT = np.concatenate([r["outT"] for r in res.results], axis=1)
    return np.ascontiguousarray(outT.T), res


def kernel(x, W1, b1, W2, b2):
    out, _ = run(x, W1, b1, W2, b2, trace=False)
    return out


# revision 9
# speedup vs baseline: 1.3195x; 1.3195x over previous
"""Trainium2 Bass kernel for nn_ODEModel (single 3/8-rule RK4 step of a
2-layer MLP ODE function), data-parallel across 8 NeuronCores.

Math (per row of x, dt=1):
    f(y) = W2 @ relu(W1 @ y + b1) + b2
    k1 = f(x); k2 = f(x + k1/3); k3 = f(x + k2 - k1/3); k4 = f(x + k1 - k2 + k3)
    out = x + (k1 + 3*(k2 + k3) + k4) / 8

Device strategy (per core, shard of B rows):
  - Transposed activation layout: tiles are [feat/hid partitions, rows free].
    Host pre-transposes x into xT [128, B] so every DMA is contiguous.
  - Per block of R=512 rows: z = W1@y via 2 bf16 matmuls (hid chunks) into a
    fused [128, 1024] PSUM tile; h = relu(z) is ONE activation op on ScalarE;
    k_s = W2@h_s via 2 accumulating matmuls; the final RK4 combination
    sum((c_s*W2) @ h_s) is accumulated in PSUM across all 4 stages with
    host-pre-scaled bf16 weight copies (c = 1/8, 3/8, 3/8, 1/8).
  - y combinations are fused scalar_tensor_tensor ops: y = (k * c) + prev.
    k1-derived ones (y2, xm, v) run on GpSimd from an SBUF copy of k1;
    the PSUM-reading ones (y3, w, y4, out) run on VectorE.
  - The emission order is software-pipelined with a 2-stage skew between
    consecutive row blocks so the RK4 serial dependency chain of one block
    hides under the engine work of its neighbors.

b1/b2 are zero in the reference's setup_inputs; a bias-correct variant
(per-chunk relus with per-partition bias, bias terms folded host-side)
is built only when a nonzero bias is actually passed.
"""

import ml_dtypes
import numpy as np

import concourse.bass as bass
import concourse.bacc as bacc_mod
import concourse.mybir as mybir
from concourse.bass_utils import run_bass_kernel_spmd
from concourse.tile import TileContext

F32 = mybir.dt.float32
BF16 = mybir.dt.bfloat16
AF = mybir.ActivationFunctionType
ALU = mybir.AluOpType

N_CORES = 8
D = 128          # IN_DIM
H = 256          # HID
R = 512          # rows per block
BATCH = 262144
B_LOCAL = BATCH // N_CORES          # 32768 rows per core
NBLK = B_LOCAL // R                 # 64 blocks per core

# y2/xm/v on GpSimd (from an SBUF k1 copy) to unload VectorE
GPSIMD_STT = False


def build(nblk: int, with_bias: bool) -> bass.Bass:
    nc = bacc_mod.Bacc(None, target_bir_lowering=False, debug=False)
    B = nblk * R

    xT = nc.declare_dram_parameter("xT", [D, B], F32, isOutput=False)
    w1t = nc.declare_dram_parameter("w1t", [D, H], BF16, isOutput=False)
    # w2ts: [0] = W2.T, [1] = W2.T/8, [2] = 3*W2.T/8   (each [H, D], bf16)
    w2ts = nc.declare_dram_parameter("w2ts", [3, H, D], BF16, isOutput=False)
    if with_bias:
        # biasesT[p, s*2+c] = (b1 + cfold[s]*(W1@b2))[c*128+p], cfold=(0,1/3,2/3,1)
        biases = nc.declare_dram_parameter("biases", [D, 8], F32, isOutput=False)
        b2col = nc.declare_dram_parameter("b2col", [D, 1], F32, isOutput=False)
    outT = nc.declare_dram_parameter("outT", [D, B], F32, isOutput=True)

    with TileContext(nc) as tc:
        with (
            tc.tile_pool(name="wpool", bufs=1) as wpool,
            tc.tile_pool(name="xpool", bufs=5) as xpool,
            tc.tile_pool(name="xbpool", bufs=4) as xbpool,
            tc.tile_pool(name="ypool", bufs=4) as ypool,
            tc.tile_pool(name="hpool", bufs=4) as hpool,
            tc.tile_pool(name="apool", bufs=3) as apool,
            tc.tile_pool(name="opool", bufs=4) as opool,
            tc.tile_pool(name="zpool", bufs=2, space="PSUM") as zpool,
            tc.tile_pool(name="kpool", bufs=2, space="PSUM") as kpool,
            tc.tile_pool(name="gpool", bufs=2, space="PSUM") as gpool,
        ):
            # ---- one-time weight/bias loads ----
            w1t_sb = wpool.tile([D, H], BF16, tag="w1t")
            nc.sync.dma_start(out=w1t_sb, in_=w1t[:, :])
            w2_sb = []  # [variant][chunk] -> [128, 128] bf16
            for v in range(3):
                per_chunk = []
                for c in range(2):
                    t = wpool.tile([D, D], BF16, tag=f"w2v{v}c{c}")
                    nc.sync.dma_start(out=t, in_=w2ts[v, c * D : (c + 1) * D, :])
                    per_chunk.append(t)
                w2_sb.append(per_chunk)
            if with_bias:
                bias_sb = wpool.tile([D, 8], F32, tag="biases")
                nc.sync.dma_start(out=bias_sb, in_=biases[:, :])
                b2_sb = wpool.tile([D, 1], F32, tag="b2col")
                nc.sync.dma_start(out=b2_sb, in_=b2col[:, :])

            # g-accumulation weight variant per stage: 1/8, 3/8, 3/8, 1/8
            gvar = (1, 2, 2, 1)

            def mlp_z(y_sb, stage):
                """z = W1 @ y; h = relu(z + bias_s). Returns h tile."""
                z = zpool.tile([D, 2 * R], F32, tag="z")
                for c in range(2):
                    nc.tensor.matmul(
                        out=z[:, c * R : (c + 1) * R],
                        lhsT=w1t_sb[:, c * D : (c + 1) * D],
                        rhs=y_sb[:, :],
                        start=True,
                        stop=True,
                    )
                h = hpool.tile([D, 2 * R], BF16, tag="h")
                if with_bias:
                    for c in range(2):
                        nc.scalar.activation(
                            out=h[:, c * R : (c + 1) * R],
                            in_=z[:, c * R : (c + 1) * R],
                            func=AF.Relu,
                            bias=bias_sb[:, stage * 2 + c : stage * 2 + c + 1],
                        )
                else:
                    nc.scalar.activation(out=h[:, :], in_=z[:, :], func=AF.Relu)
                return h

            def mm_k(h, stage, g, want_k):
                """k_s = W2 @ h_s (PSUM accum over hid chunks) if want_k,
                plus g += (c_s * W2) @ h_s into the block's g accumulator."""
                k = None
                if want_k:
                    k = kpool.tile([D, R], F32, tag="k")
                    for c in range(2):
                        nc.tensor.matmul(
                            out=k[:, :],
                            lhsT=w2_sb[0][c][:, :],
                            rhs=h[:, c * R : (c + 1) * R],
                            start=(c == 0),
                            stop=(c == 1),
                        )
                for c in range(2):
                    nc.tensor.matmul(
                        out=g[:, :],
                        lhsT=w2_sb[gvar[stage]][c][:, :],
                        rhs=h[:, c * R : (c + 1) * R],
                        start=(stage == 0 and c == 0),
                        stop=(stage == 3 and c == 1),
                        skip_group_check=True,
                    )
                return k

            def stt(eng, out_ap, in0, scalar, in1):
                eng.scalar_tensor_tensor(
                    out=out_ap,
                    in0=in0,
                    scalar=scalar,
                    in1=in1,
                    op0=ALU.mult,
                    op1=ALU.add,
                )

            # ---- per-block state and stage closures (for the skew) ----
            st: list[dict] = [dict() for _ in range(nblk)]

            def pre(i):
                s = st[i]
                cols = slice(i * R, (i + 1) * R)
                s["cols"] = cols
                x_t = xpool.tile([D, R], F32, tag="x")
                nc.sync.dma_start(out=x_t, in_=xT[:, cols])
                x_bf = xbpool.tile([D, R], BF16, tag="xbf")
                nc.vector.tensor_copy(out=x_bf[:, :], in_=x_t[:, :])
                s["x"] = x_t
                s["xbf"] = x_bf

            def s1(i):
                s = st[i]
                g = gpool.tile([D, R], F32, tag="g")
                s["g"] = g
                h1 = mlp_z(s["xbf"], 0)
                k1 = mm_k(h1, 0, g, True)
                # k1 combos
                y2 = ypool.tile([D, R], BF16, tag="y")
                xm = apool.tile([D, R], BF16, tag="xm")
                v = apool.tile([D, R], BF16, tag="v")
                if GPSIMD_STT:
                    # k1 scaled copies on DVE, plain adds on GpSimd
                    k1e3 = apool.tile([D, R], BF16, tag="k1e3")
                    nc.vector.tensor_scalar_mul(k1e3[:, :], k1[:, :], 1.0 / 3.0)
                    k1e = apool.tile([D, R], BF16, tag="k1e")
                    nc.vector.tensor_copy(out=k1e[:, :], in_=k1[:, :])
                    nc.gpsimd.tensor_tensor(
                        y2[:, :], s["xbf"][:, :], k1e3[:, :], ALU.add
                    )
                    nc.gpsimd.tensor_tensor(
                        xm[:, :], s["xbf"][:, :], k1e3[:, :], ALU.subtract
                    )
                    nc.gpsimd.tensor_tensor(
                        v[:, :], s["xbf"][:, :], k1e[:, :], ALU.add
                    )
                else:
                    stt(nc.vector, y2[:, :], k1[:, :], 1.0 / 3.0, s["x"][:, :])
                    stt(nc.vector, xm[:, :], k1[:, :], -1.0 / 3.0, s["x"][:, :])
                    stt(nc.vector, v[:, :], k1[:, :], 1.0, s["x"][:, :])
                s["y2"], s["xm"], s["v"] = y2, xm, v

            def s2(i):
                s = st[i]
                h2 = mlp_z(s["y2"], 1)
                k2 = mm_k(h2, 1, s["g"], True)
                y3 = ypool.tile([D, R], BF16, tag="y")
                stt(nc.vector, y3[:, :], k2[:, :], 1.0, s["xm"][:, :])
                w = apool.tile([D, R], BF16, tag="w")
                stt(nc.vector, w[:, :], k2[:, :], -1.0, s["v"][:, :])
                s["y3"], s["w"] = y3, w

            def s3(i):
                s = st[i]
                h3 = mlp_z(s["y3"], 2)
                k3 = mm_k(h3, 2, s["g"], True)
                y4 = ypool.tile([D, R], BF16, tag="y")
                stt(nc.vector, y4[:, :], k3[:, :], 1.0, s["w"][:, :])
                s["y4"] = y4

            def s4(i):
                s = st[i]
                h4 = mlp_z(s["y4"], 3)
                mm_k(h4, 3, s["g"], False)
                o = opool.tile([D, R], F32, tag="o")
                if with_bias:
                    nc.vector.scalar_tensor_tensor(
                        out=o[:, :],
                        in0=s["g"][:, :],
                        scalar=b2_sb[:, :],
                        in1=s["x"][:, :],
                        op0=ALU.add,
                        op1=ALU.add,
                    )
                else:
                    stt(nc.vector, o[:, :], s["g"][:, :], 1.0, s["x"][:, :])
                nc.sync.dma_start(out=outT[:, s["cols"]], in_=o[:, :])
                st[i] = {}  # release references

            # ---- skewed emission: 2-stage offset between adjacent blocks ----
            # order: ..., S3(i), S1(i+1), S4(i), S2(i+1), S3(i+1), S1(i+2), ...
            pre(0)
            s1(0)
            if nblk > 1:
                pre(1)
            s2(0)
            if nblk == 1:
                s3(0)
                s4(0)
            else:
                for i in range(nblk - 1):
                    s3(i)
                    s1(i + 1)
                    if i + 2 < nblk:
                        pre(i + 2)
                    s4(i)
                    s2(i + 1)
                s3(nblk - 1)
                s4(nblk - 1)

    nc.finalize()
    return nc


_cache: dict = {}


def _get_nc(nblk: int, with_bias: bool) -> bass.Bass:
    key = (nblk, with_bias)
    if key not in _cache:
        _cache[key] = build(nblk, with_bias)
    return _cache[key]


def _prep_host(x, W1, b1, W2, b2):
    x = np.asarray(x, dtype=np.float32)
    W1 = np.asarray(W1, dtype=np.float32)
    b1 = np.asarray(b1, dtype=np.float32)
    W2 = np.asarray(W2, dtype=np.float32)
    b2 = np.asarray(b2, dtype=np.float32)

    with_bias = bool(np.any(b1) or np.any(b2))

    xT = np.ascontiguousarray(x.T)                                 # [128, BATCH]
    w1t = np.ascontiguousarray(W1.T).astype(ml_dtypes.bfloat16)    # [128, 256]
    w2t = np.ascontiguousarray(W2.T)                               # [256, 128]
    w2ts = np.stack([w2t, w2t / 8.0, 3.0 * w2t / 8.0]).astype(
        ml_dtypes.bfloat16
    )  # [3, 256, 128]

    shard_maps = []
    for c in range(N_CORES):
        m = {
            "xT": np.ascontiguousarray(xT[:, c * B_LOCAL : (c + 1) * B_LOCAL]),
            "w1t": w1t,
            "w2ts": w2ts,
        }
        if with_bias:
            c1 = W1 @ b2
            bl = np.stack(
                [b1, b1 + c1 / 3.0, b1 + 2.0 * c1 / 3.0, b1 + c1]
            ).astype(np.float32)  # [4, 256]
            # -> [128, 8]: col s*2+c holds bias_s[c*128:(c+1)*128]
            m["biases"] = np.ascontiguousarray(
                bl.reshape(4, 2, D).transpose(2, 0, 1).reshape(D, 8)
            )
            m["b2col"] = b2.reshape(D, 1)
        shard_maps.append(m)
    return shard_maps, with_bias


def run(x, W1, b1, W2, b2, trace: bool = False):
    """Run on the 8 cores; returns (out [BATCH, 128] fp32, BassKernelResults)."""
    shard_maps, with_bias = _prep_host(x, W1, b1, W2, b2)
    nc = _get_nc(NBLK, with_bias)
    res = run_bass_kernel_spmd(
        nc, shard_maps, core_ids=list(range(N_CORES)), trace=trace
    )
    outT = np.concatenate([r["outT"] for r in res.results], axis=1)
    return np.ascontiguousarray(outT.T), res


def kernel(x, W1, b1, W2, b2):
    out, _ = run(x, W1, b1, W2, b2, trace=False)
    return out


# revision 12
# speedup vs baseline: 1.9183x; 1.4538x over previous
"""Trainium2 Bass kernel for nn_ODEModel (single 3/8-rule RK4 step of a
2-layer MLP ODE function), data-parallel across 8 NeuronCores.

Math (per row of x, dt=1):
    f(y) = W2 @ relu(W1 @ y + b1) + b2
    k1 = f(x); k2 = f(x + k1/3); k3 = f(x + k2 - k1/3); k4 = f(x + k1 - k2 + k3)
    out = x + (k1 + 3*(k2 + k3) + k4) / 8

Device strategy (per core, shard of B rows):
  - Transposed activation layout: tiles are [feat/hid partitions, rows free].
    Host pre-transposes x into xT [128, B] so every DMA is contiguous.
  - Per block of R=512 rows: z = W1@y via 2 bf16 matmuls (hid chunks) into a
    fused [128, 1024] PSUM tile; h = relu(z) is ONE activation op on ScalarE;
    k_s = W2@h_s via 2 accumulating matmuls; the final RK4 combination
    sum((c_s*W2) @ h_s) is accumulated in PSUM across all 4 stages with
    host-pre-scaled bf16 weight copies (c = 1/8, 3/8, 3/8, 1/8).
  - y combinations are fused scalar_tensor_tensor ops: y = (k * c) + prev.
    k1-derived ones (y2, xm, v) run on GpSimd from an SBUF copy of k1;
    the PSUM-reading ones (y3, w, y4, out) run on VectorE.
  - The emission order is software-pipelined with a 2-stage skew between
    consecutive row blocks so the RK4 serial dependency chain of one block
    hides under the engine work of its neighbors.

b1/b2 are zero in the reference's setup_inputs; a bias-correct variant
(per-chunk relus with per-partition bias, bias terms folded host-side)
is built only when a nonzero bias is actually passed.
"""

import ml_dtypes
import numpy as np

import concourse.bass as bass
import concourse.bacc as bacc_mod
import concourse.mybir as mybir
from concourse.bass_utils import run_bass_kernel_spmd
from concourse.tile import TileContext

F32 = mybir.dt.float32
BF16 = mybir.dt.bfloat16
AF = mybir.ActivationFunctionType
ALU = mybir.AluOpType

N_CORES = 8
D = 128          # IN_DIM
H = 256          # HID
R = 512          # rows per block
BATCH = 262144
B_LOCAL = BATCH // N_CORES          # 32768 rows per core
NBLK = B_LOCAL // R                 # 64 blocks per core

# y2/xm/v on GpSimd (from an SBUF k1 copy) to unload VectorE
GPSIMD_STT = False


def build(nblk: int, with_bias: bool) -> bass.Bass:
    nc = bacc_mod.Bacc(None, target_bir_lowering=False, debug=False)
    B = nblk * R

    xT = nc.declare_dram_parameter("xT", [D, B], F32, isOutput=False)
    w1t = nc.declare_dram_parameter("w1t", [D, H], BF16, isOutput=False)
    # w2ts: [0] = W2.T, [1] = W2.T/8, [2] = 3*W2.T/8   (each [H, D], bf16)
    w2ts = nc.declare_dram_parameter("w2ts", [3, H, D], BF16, isOutput=False)
    if with_bias:
        # biasesT[p, s*2+c] = (b1 + cfold[s]*(W1@b2))[c*128+p], cfold=(0,1/3,2/3,1)
        biases = nc.declare_dram_parameter("biases", [D, 8], F32, isOutput=False)
        b2col = nc.declare_dram_parameter("b2col", [D, 1], F32, isOutput=False)
    outT = nc.declare_dram_parameter("outT", [D, B], F32, isOutput=True)

    with TileContext(nc) as tc:
        with (
            tc.tile_pool(name="wpool", bufs=1) as wpool,
            tc.tile_pool(name="xpool", bufs=5) as xpool,
            tc.tile_pool(name="xbpool", bufs=4) as xbpool,
            tc.tile_pool(name="ypool", bufs=4) as ypool,
            tc.tile_pool(name="hpool", bufs=4) as hpool,
            tc.tile_pool(name="apool", bufs=3) as apool,
            tc.tile_pool(name="ppool", bufs=3) as ppool,
            tc.tile_pool(name="opool", bufs=4) as opool,
            tc.tile_pool(name="zpool", bufs=2, space="PSUM") as zpool,
            tc.tile_pool(name="kpool", bufs=2, space="PSUM") as kpool,
            tc.tile_pool(name="gpool", bufs=2, space="PSUM") as gpool,
        ):
            # ---- one-time weight/bias loads ----
            w1t_sb = wpool.tile([D, H], BF16, tag="w1t")
            nc.sync.dma_start(out=w1t_sb, in_=w1t[:, :])
            w2_sb = []  # [variant][chunk] -> [128, 128] bf16
            for v in range(3):
                per_chunk = []
                for c in range(2):
                    t = wpool.tile([D, D], BF16, tag=f"w2v{v}c{c}")
                    nc.sync.dma_start(out=t, in_=w2ts[v, c * D : (c + 1) * D, :])
                    per_chunk.append(t)
                w2_sb.append(per_chunk)
            if with_bias:
                bias_sb = wpool.tile([D, 8], F32, tag="biases")
                nc.sync.dma_start(out=bias_sb, in_=biases[:, :])
                b2_sb = wpool.tile([D, 1], F32, tag="b2col")
                nc.sync.dma_start(out=b2_sb, in_=b2col[:, :])

            # g-accumulation weight variant per stage: 1/8, 3/8, 3/8, 1/8
            gvar = (1, 2, 2, 1)

            def mlp_z(y_sb, stage):
                """z = W1 @ y; h = relu(z + bias_s). Returns h tile."""
                z = zpool.tile([D, 2 * R], F32, tag="z")
                for c in range(2):
                    nc.tensor.matmul(
                        out=z[:, c * R : (c + 1) * R],
                        lhsT=w1t_sb[:, c * D : (c + 1) * D],
                        rhs=y_sb[:, :],
                        start=True,
                        stop=True,
                    )
                h = hpool.tile([D, 2 * R], BF16, tag="h")
                if with_bias:
                    for c in range(2):
                        nc.scalar.activation(
                            out=h[:, c * R : (c + 1) * R],
                            in_=z[:, c * R : (c + 1) * R],
                            func=AF.Relu,
                            bias=bias_sb[:, stage * 2 + c : stage * 2 + c + 1],
                        )
                else:
                    nc.scalar.activation(out=h[:, :], in_=z[:, :], func=AF.Relu)
                return h

            def mm_k(h, stage, g, want_k, start, stop, out_k):
                """k_s = W2 @ h_s (PSUM accum over hid chunks) if want_k,
                plus g += (c_s * W2) @ h_s into a half-block g accumulator."""
                if want_k:
                    k = kpool.tile([D, R], F32, tag="k")
                    for c in range(2):
                        nc.tensor.matmul(
                            out=k[:, :],
                            lhsT=w2_sb[0][c][:, :],
                            rhs=h[:, c * R : (c + 1) * R],
                            start=(c == 0),
                            stop=(c == 1),
                        )
                    out_k["k"] = k
                for c in range(2):
                    nc.tensor.matmul(
                        out=g[:, :],
                        lhsT=w2_sb[gvar[stage]][c][:, :],
                        rhs=h[:, c * R : (c + 1) * R],
                        start=(start and c == 0),
                        stop=(stop and c == 1),
                        skip_group_check=True,
                    )

            def stt(eng, out_ap, in0, scalar, in1):
                eng.scalar_tensor_tensor(
                    out=out_ap,
                    in0=in0,
                    scalar=scalar,
                    in1=in1,
                    op0=ALU.mult,
                    op1=ALU.add,
                )

            # ---- per-block state and stage closures (for the skew) ----
            st: list[dict] = [dict() for _ in range(nblk)]

            def pre(i):
                s = st[i]
                cols = slice(i * R, (i + 1) * R)
                s["cols"] = cols
                x_t = xpool.tile([D, R], F32, tag="x")
                nc.sync.dma_start(out=x_t, in_=xT[:, cols])
                # bf16 copy of x via casting SWDGE DMA (GpSimd sequencer)
                x_bf = xbpool.tile([D, R], BF16, tag="xbf")
                nc.gpsimd.dma_start(out=x_bf[:, :], in_=xT[:, cols])
                s["x"] = x_t
                s["xbf"] = x_bf

            def s1(i):
                s = st[i]
                g12 = gpool.tile([D, R], F32, tag="g")
                s["g12"] = g12
                h1 = mlp_z(s["xbf"], 0)
                mm_k(h1, 0, g12, True, start=True, stop=False, out_k=s)
                k1 = s["k"]
                xb = s["xbf"]
                # y2 = x + k1/3 ; xm = 2x - y2 ; v = 3*y2 - 2x (= x + k1)
                y2 = ypool.tile([D, R], BF16, tag="y")
                stt(nc.vector, y2[:, :], k1[:, :], 1.0 / 3.0, xb[:, :])
                xm = apool.tile([D, R], BF16, tag="xm")
                nc.vector.scalar_tensor_tensor(
                    out=xm[:, :], in0=xb[:, :], scalar=2.0, in1=y2[:, :],
                    op0=ALU.mult, op1=ALU.subtract,
                )
                t3 = apool.tile([D, R], BF16, tag="t3")
                nc.vector.tensor_scalar_mul(t3[:, :], y2[:, :], 3.0)
                v = apool.tile([D, R], BF16, tag="v")
                nc.vector.scalar_tensor_tensor(
                    out=v[:, :], in0=xb[:, :], scalar=-2.0, in1=t3[:, :],
                    op0=ALU.mult, op1=ALU.add,
                )
                s["y2"], s["xm"], s["v"] = y2, xm, v

            def s2(i):
                s = st[i]
                h2 = mlp_z(s["y2"], 1)
                mm_k(h2, 1, s["g12"], True, start=False, stop=True, out_k=s)
                k2 = s["k"]
                y3 = ypool.tile([D, R], BF16, tag="y")
                stt(nc.vector, y3[:, :], k2[:, :], 1.0, s["xm"][:, :])
                w = apool.tile([D, R], BF16, tag="w")
                stt(nc.vector, w[:, :], k2[:, :], -1.0, s["v"][:, :])
                # partial output: o12 = x + (k1 + 3*k2)/8
                o12 = ppool.tile([D, R], F32, tag="o12")
                stt(nc.vector, o12[:, :], s["g12"][:, :], 1.0, s["x"][:, :])
                s["y3"], s["w"], s["o12"] = y3, w, o12

            def s3(i):
                s = st[i]
                g34 = gpool.tile([D, R], F32, tag="g")
                s["g34"] = g34
                h3 = mlp_z(s["y3"], 2)
                mm_k(h3, 2, g34, True, start=True, stop=False, out_k=s)
                k3 = s["k"]
                y4 = ypool.tile([D, R], BF16, tag="y")
                stt(nc.vector, y4[:, :], k3[:, :], 1.0, s["w"][:, :])
                s["y4"] = y4

            def s4(i):
                s = st[i]
                h4 = mlp_z(s["y4"], 3)
                mm_k(h4, 3, s["g34"], False, start=False, stop=True, out_k=s)
                o = opool.tile([D, R], F32, tag="o")
                if with_bias:
                    nc.vector.scalar_tensor_tensor(
                        out=o[:, :],
                        in0=s["g34"][:, :],
                        scalar=b2_sb[:, :],
                        in1=s["o12"][:, :],
                        op0=ALU.add,
                        op1=ALU.add,
                    )
                else:
                    stt(nc.vector, o[:, :], s["g34"][:, :], 1.0, s["o12"][:, :])
                nc.sync.dma_start(out=outT[:, s["cols"]], in_=o[:, :])
                st[i] = {}  # release references

            # ---- stage-granular software pipeline: 4 blocks in flight ----
            pre(0)
            for j in range(nblk + 3):
                if j + 1 < nblk:
                    pre(j + 1)
                if 0 <= j - 3 < nblk:
                    s4(j - 3)
                if 0 <= j - 2 < nblk:
                    s3(j - 2)
                if 0 <= j - 1 < nblk:
                    s2(j - 1)
                if j < nblk:
                    s1(j)

    nc.finalize()
    return nc


_cache: dict = {}


def _get_nc(nblk: int, with_bias: bool) -> bass.Bass:
    key = (nblk, with_bias)
    if key not in _cache:
        _cache[key] = build(nblk, with_bias)
    return _cache[key]


def _prep_host(x, W1, b1, W2, b2):
    x = np.asarray(x, dtype=np.float32)
    W1 = np.asarray(W1, dtype=np.float32)
    b1 = np.asarray(b1, dtype=np.float32)
    W2 = np.asarray(W2, dtype=np.float32)
    b2 = np.asarray(b2, dtype=np.float32)

    with_bias = bool(np.any(b1) or np.any(b2))

    xT = np.ascontiguousarray(x.T)                                 # [128, BATCH]
    w1t = np.ascontiguousarray(W1.T).astype(ml_dtypes.bfloat16)    # [128, 256]
    w2t = np.ascontiguousarray(W2.T)                               # [256, 128]
    w2ts = np.stack([w2t, w2t / 8.0, 3.0 * w2t / 8.0]).astype(
        ml_dtypes.bfloat16
    )  # [3, 256, 128]

    shard_maps = []
    for c in range(N_CORES):
        m = {
            "xT": np.ascontiguousarray(xT[:, c * B_LOCAL : (c + 1) * B_LOCAL]),
            "w1t": w1t,
            "w2ts": w2ts,
        }
        if with_bias:
            c1 = W1 @ b2
            bl = np.stack(
                [b1, b1 + c1 / 3.0, b1 + 2.0 * c1 / 3.0, b1 + c1]
            ).astype(np.float32)  # [4, 256]
            # -> [128, 8]: col s*2+c holds bias_s[c*128:(c+1)*128]
            m["biases"] = np.ascontiguousarray(
                bl.reshape(4, 2, D).transpose(2, 0, 1).reshape(D, 8)
            )
            m["b2col"] = b2.reshape(D, 1)
        shard_maps.append(m)
    return shard_maps, with_bias


def run(x, W1, b1, W2, b2, trace: bool = False):
    """Run on the 8 cores; returns (out [BATCH, 128] fp32, BassKernelResults)."""
    shard_maps, with_bias = _prep_host(x, W1, b1, W2, b2)
    nc = _get_nc(NBLK, with_bias)
    res = run_bass_kernel_spmd(
        nc, shard_maps, core_ids=list(range(N_CORES)), trace=trace
    )
    outT = np.concatenate([r["outT"] for r in res.results], axis=1)
    return np.ascontiguousarray(outT.T), res


def kernel(x, W1, b1, W2, b2):
    out, _ = run(x, W1, b1, W2, b2, trace=False)
    return out
